# revision 25
# baseline (speedup 1.0000x reference)
"""Trainium2 Bass kernel for banded continuous-conv1d (sparse_attention).

Math (per batch b, position i, K=16 band offsets d=1..K):
    dt[b,i,d] = relu(t_i - t_{i-d})           (masked where i-d < 0)
    h1 = relu(dt @ W1 + b1)                   (scalar -> 128)
    h2 = relu(h1 @ W2 + b2)                   (128 -> 128)
    kv = (h2 @ W3 + b3) masked                (128 -> 32*32)
    out[b,i,o] = sum_{d,c} feat[b,i,c] * kv[b,i,d,c,o]

Fast path (zero biases, the spec's declared fill): times are sorted so
dt >= 0 always, and relu is positively homogeneous, so the whole MLP
collapses to a rank-1 map:
    h1 = dt * relu(W1);  h2 = relu(dt * (relu(W1) @ W2)) = dt * relu(relu(W1) @ W2)
    kv = dt * w,  w = relu(relu(W1) @ W2) @ W3          (host-precomputed)
    out[b,i,o] = S[b,i] * (feat[b,i,:] @ Wt)[o]
where S[b,i] = sum_d valid * (t_i - t_{i-d}) and Wt = w.reshape(32,32).
The device computes S (banded time-sum) and feat @ Wt, then scales.
Masking is folded into the staged tA/tB tensors (zeros at invalid
entries), so no relu/mask ops are needed on device.

Nonzero biases fall back to the full per-offset MLP pipeline below
(phase-separated matmuls with rank-1 bias/mask corrections).

Sharding: 8 cores = 2 batches x 4 sequence shards of 512 positions.
"""

import sys

import numpy as np

sys.path.insert(0, "/opt/trn_rl_repo")

from concourse import bacc, bass, mybir, tile  # noqa: E402
from concourse.bass_utils import run_bass_kernel_spmd  # noqa: E402

BS, L, CIN, COUT, HID, K = 2, 2048, 32, 32, 128, 16
NCORES = 8
NSH = 4          # sequence shards per batch
SH = L // NSH    # positions per core (512)
NQT = SH // 128  # q-tiles per core (4)
F32 = mybir.dt.float32

_cache: dict = {}


def _enable_ldw_opt():
    """Let walrus dedup identical consecutive LDWEIGHTS (the default
    --enable-ldw-opt=false re-loads the stationary operand before every
    matmul)."""
    from concourse import bass_utils

    if getattr(bass_utils.run_command, "_ldw_patched", False):
        return
    orig = bass_utils.run_command

    def patched(cmd, *a, **kw):
        cmd = [
            c.replace("--enable-ldw-opt=false", "--enable-ldw-opt=true")
            if isinstance(c, str) else c
            for c in cmd
        ]
        return orig(cmd, *a, **kw)

    patched._ldw_patched = True
    bass_utils.run_command = patched


def _build_fast():
    """Collapsed zero-bias kernel: out = S * (feat @ Wt) per position.

    Note: no _enable_ldw_opt here — fp16 InstLdweights is incompatible
    with walrus --enable-ldw-opt, and the 4 matmuls all have distinct
    stationary operands so dedup buys nothing."""
    # Skip the const-pool all_engine_barrier emitted by Bass.__init__ (we
    # never read the const APs, and the runtime entry barrier already
    # synchronizes the engines) — it stalls every queue ~0.7us before the
    # first input DMA can issue. Scoped to construction only; TileContext
    # teardown barriers are untouched.
    orig_barrier = bass.Bass.all_engine_barrier

    def _skip_barrier(self, *, sem_only=False):
        pass

    bass.Bass.all_engine_barrier = _skip_barrier
    try:
        nc = bacc.Bacc("TRN2", target_bir_lowering=False, debug=False)
    finally:
        bass.Bass.all_engine_barrier = orig_barrier

    F16 = mybir.dt.float16
    # tS[q', t*17+d] = -t_{i-1-d} * valid  (d<16),  t_i * nv(i)  (d=16),
    # so S[q', t] is a single segmented reduce.
    # featW[c, 0:512] = features transposed; [c, 512:544] = Wt
    dram = {
        "tS": nc.dram_tensor("tS", [128, NQT * (K + 1)], F32,
                             kind="ExternalInput"),
        "featW": nc.dram_tensor("featW", [CIN, SH + COUT], F16,
                                kind="ExternalInput"),
    }
    # packed partition-major: out[q', t*32+o]; host unshard untangles it
    out_dram = nc.dram_tensor("out", [128, NQT * COUT], F16, kind="ExternalOutput")

    Add = mybir.AluOpType.add
    Mult = mybir.AluOpType.mult

    with tile.TileContext(nc) as tc:
        with (
            tc.tile_pool(name="const", bufs=1) as const,
            tc.tile_pool(name="work", bufs=1) as work,
            tc.tile_pool(name="ps", bufs=1, space=bass.MemorySpace.PSUM) as ps,
        ):
            qeng = {"featW": nc.sync, "tS": nc.scalar}
            sb = {}
            for name in dram:
                t = const.tile(list(dram[name].shape), dram[name].dtype, tag=name)
                qeng[name].dma_start(t[:], dram[name].ap())
                sb[name] = t

            # S[q', t] = t_i*nv - sum_d t_{i-1-d}*valid  (signs pre-staged)
            S = work.tile([128, NQT], F32, tag="S")
            nc.vector.tensor_reduce(
                S[:],
                sb["tS"][:].rearrange("p (t d) -> p t d", d=K + 1),
                axis=mybir.AxisListType.X,
                op=Add,
            )

            # fw[q, t*32+o] = (feat @ Wt)[q-tile t]; all 4 in one PSUM bank
            kvp = ps.tile([128, 512], F32, tag="kv")
            for t in range(NQT):
                qs = slice(t * 128, (t + 1) * 128)
                nc.tensor.matmul(
                    kvp[:, t * COUT : (t + 1) * COUT],
                    sb["featW"][:, qs], sb["featW"][:, SH:],
                    start=True, stop=True,
                )
            # out = S * fw in one broadcast multiply, then one packed DMA
            ot = work.tile([128, NQT * COUT], F16, tag="ot")
            nc.vector.tensor_tensor(
                ot[:].rearrange("p (t o) -> p t o", o=COUT),
                kvp[:, : NQT * COUT].rearrange("p (t o) -> p t o", o=COUT),
                S[:].unsqueeze(2).broadcast_to([128, NQT, COUT]),
                op=Mult,
            )
            nc.scalar.dma_start(out_dram.ap(), ot[:])

    nc.compile()
    return nc


def _stage_fast(times, features, W1, W2, W3):
    times = np.ascontiguousarray(times, dtype=np.float32)
    features = np.ascontiguousarray(features, dtype=np.float32)
    w = np.maximum(np.maximum(W1.reshape(-1), 0.0) @ W2, 0.0) @ W3
    Wt = np.ascontiguousarray(w.reshape(CIN, COUT), np.float32)
    dd = np.arange(K)

    in_maps = []
    for c in range(NCORES):
        b, s = divmod(c, NSH)
        pos = s * SH + np.arange(SH).reshape(NQT, 128)      # [t, q']
        src = pos[:, :, None] - 1 - dd[None, None, :]       # [t, q', d]
        valid = src >= 0
        tqnv = times[b, pos] * valid.sum(-1)                # [t, q']
        tBm = -(times[b, np.clip(src, 0, L - 1)] * valid)   # [t, q', d]
        tS = (
            np.concatenate([tBm, tqnv[:, :, None]], axis=2)  # [t, q', 17]
            .transpose(1, 0, 2).reshape(128, NQT * (K + 1))
        )
        featW = np.concatenate(
            [features[b, s * SH : (s + 1) * SH].T, Wt], axis=1
        )
        in_maps.append({
            "tS": np.ascontiguousarray(tS, np.float32),
            "featW": np.ascontiguousarray(featW.astype(np.float16)),
        })
    return in_maps


# ---------------------------------------------------------------------------
# Full per-offset MLP pipeline (fallback for nonzero biases).
# ---------------------------------------------------------------------------

def _build_bass(with_corr):
    """Build + compile the SPMD single-core Bass program (identical on all
    cores; per-core behavior comes entirely from the input tensors)."""
    _enable_ldw_opt()
    nc = bacc.Bacc("TRN2", target_bir_lowering=False, debug=False)

    R32 = mybir.dt.float32r  # fp32 bits, single-pass PE mode (1 cyc/row vs 4)
    specs = [
        ("tA", (K, SH), F32),       # t_i broadcast over d rows
        ("tB", (K, SH), F32),       # t_{i-1-d}, halo-padded (clipped to t_0)
        ("mask16", (K, SH), F32),   # 1.0 where i-1-d >= 0
        ("featq", (128, NQT * CIN), F32),  # feat[q, t*32+c] (q-tile-major)
        ("W1r", (1, HID), R32),     # W1 row
        ("W2", (HID, HID), R32),
        ("W3", (HID, CIN * COUT), R32),
        ("b1c", (HID, 1), F32),
        ("b2c", (HID, 1), F32),
        ("eye", (HID, HID), R32),   # identity for the d-sum PSUM accumulation
    ]
    if with_corr:
        specs += [
            ("nvmat", (2, SH), R32),    # rows: nv, K-nv (valid-offset counts)
            ("rhs2", (2, CIN * COUT), R32),  # rows: b3, -kv0
        ]
    dram = {}
    for name, shape, dt_ in specs:
        dram[name] = nc.dram_tensor(name, list(shape), dt_, kind="ExternalInput")
    out_dram = nc.dram_tensor("out", [SH, COUT], F32, kind="ExternalOutput")

    Relu = mybir.ActivationFunctionType.Relu
    Add = mybir.AluOpType.add
    Max = mybir.AluOpType.max
    Mult = mybir.AluOpType.mult

    NW = 1024  # wide tile: 2 d-offsets side by side (2 PSUM banks)

    with tile.TileContext(nc) as tc:
        with (
            tc.tile_pool(name="const", bufs=1) as const,
            tc.tile_pool(name="work", bufs=1) as work,
            tc.tile_pool(name="h1p", bufs=8) as h1p,
            tc.tile_pool(name="h2p", bufs=8) as h2p,
            tc.tile_pool(name="stage5", bufs=2) as s5p,
            tc.tile_pool(name="ps1", bufs=2, space=bass.MemorySpace.PSUM) as ps1,
            tc.tile_pool(name="ps2", bufs=2, space=bass.MemorySpace.PSUM) as ps2,
        ):
            wzf = work.tile([HID, SH], F32, tag="wzf")
            nc.vector.memset(wzf[:], 0.0)
            wz = work.tile([HID, SH], R32, tag="wz")
            nc.vector.tensor_copy(wz[:], wzf[:])

            qeng = {
                "tA": nc.sync, "tB": nc.sync, "mask16": nc.sync,
                "W1r": nc.scalar, "b1c": nc.scalar, "W2": nc.scalar,
                "b2c": nc.scalar, "eye": nc.scalar,
                "W3": nc.gpsimd, "featq": nc.gpsimd,
                "nvmat": nc.gpsimd, "rhs2": nc.gpsimd,
            }
            sb = {}
            for name in dram:
                t = const.tile(list(dram[name].shape), dram[name].dtype, tag=name)
                qeng[name].dma_start(t[:], dram[name].ap())
                sb[name] = t

            for i in range(12):
                pw = ps1.tile([HID, NW], F32, tag="p1")
                nc.tensor.matmul(
                    pw[:, :SH], wz[:, :HID], wz[:], start=True, stop=True
                )

            dtsub = work.tile([K, SH], F32, tag="dtsub")
            nc.vector.tensor_sub(dtsub[:], sb["tA"][:], sb["tB"][:])
            dt2 = work.tile([K, SH], R32, tag="dt2")
            nc.vector.scalar_tensor_tensor(
                dt2[:], dtsub[:], 0.0, sb["mask16"][:], op0=Max, op1=Mult
            )
            dtrow = work.tile([1, K * SH], R32, tag="dtrow")
            nc.sync.dma_start(
                dtrow[:].rearrange("p (d q) -> p d q", d=K), dt2[:, :]
            )
            drows = [dtrow[:, d * SH : (d + 1) * SH] for d in range(K)]

            fexps = []
            for t in range(NQT):
                fe = s5p.tile([128, CIN * COUT], F32, tag=f"fe{t}")
                nc.gpsimd.tensor_copy(
                    fe[:].rearrange("p (o c) -> p o c", c=CIN),
                    sb["featq"][:, t * CIN : (t + 1) * CIN]
                    .unsqueeze(1)
                    .broadcast_to([128, COUT, CIN]),
                )
                fexps.append(fe)

            h1s = []
            for p in range(K // 2):
                pA = ps1.tile([HID, NW], F32, tag="p1")
                for j in range(2):
                    nc.tensor.matmul(
                        pA[:, j * SH : (j + 1) * SH], sb["W1r"][:],
                        drows[2 * p + j], start=True, stop=True,
                    )
                h1 = h1p.tile([HID, NW], R32, tag="h1")
                nc.scalar.activation(
                    h1[:, :SH], pA[:, :SH], Relu, bias=sb["b1c"][:]
                )
                nc.vector.tensor_scalar(
                    h1[:, SH:], pA[:, SH:], sb["b1c"][:], 0.0, op0=Add, op1=Max
                )
                h1s.append(h1)
            h2s = []
            for p in range(K // 2):
                pB = ps2.tile([HID, NW], F32, tag="p2")
                for j in range(2):
                    nc.tensor.matmul(
                        pB[:, j * SH : (j + 1) * SH], sb["W2"][:],
                        h1s[p][:, j * SH : (j + 1) * SH], start=True, stop=True,
                    )
                h2 = h2p.tile([HID, NW], R32, tag="h2")
                nc.vector.tensor_scalar(
                    h2[:, :SH], pB[:, :SH], sb["b2c"][:], 0.0, op0=Add, op1=Max
                )
                nc.scalar.activation(
                    h2[:, SH:], pB[:, SH:], Relu, bias=sb["b2c"][:]
                )
                h2s.append(h2)
            pHw = ps1.tile([HID, NW], F32, tag="p1")
            pH = pHw[:, :SH]
            n = 0
            for p in range(K // 2):
                for j in range(2):
                    nc.tensor.matmul(
                        pH, sb["eye"][:], h2s[p][:, j * SH : (j + 1) * SH],
                        start=(n == 0), stop=(n == K - 1),
                    )
                    n += 1

            Hs = work.tile([HID, SH], R32, tag="Hs")
            nc.vector.tensor_copy(Hs[:], pH)

            CO = CIN * COUT
            for t in range(NQT):
                qs = slice(t * 128, (t + 1) * 128)
                kv = (ps2 if t % 2 == 0 else ps1).tile(
                    [128, CO], F32, tag="p2" if t % 2 == 0 else "p1"
                )
                for half in range(2):
                    hs = slice(half * 512, half * 512 + 512)
                    nc.tensor.matmul(
                        kv[:, hs], Hs[:, qs], sb["W3"][:, hs],
                        start=True, stop=not with_corr,
                    )
                if with_corr:
                    for half in range(2):
                        hs = slice(half * 512, half * 512 + 512)
                        nc.tensor.matmul(
                            kv[:, hs], sb["nvmat"][:, qs], sb["rhs2"][:, hs],
                            start=False, stop=True,
                        )
                prod = s5p.tile([128, CO], F32, tag="prod")
                kvT = kv[:].rearrange("p (c o) -> p o c", o=COUT)
                prodv = prod[:].rearrange("p (o c) -> p o c", c=CIN)
                fev = fexps[t][:].rearrange("p (o c) -> p o c", c=CIN)
                if t < 1:
                    kvs = s5p.tile([128, CO], F32, tag="kvs")
                    nc.scalar.copy(kvs[:], kv[:])
                    nc.gpsimd.tensor_tensor(
                        prodv,
                        kvs[:].rearrange("p (c o) -> p o c", o=COUT),
                        fev, op=Mult,
                    )
                else:
                    nc.vector.tensor_tensor(prodv, kvT, fev, op=Mult)
                ot = s5p.tile([128, COUT], F32, tag="ot")
                nc.vector.tensor_reduce(
                    ot[:],
                    prod[:].rearrange("p (o c) -> p o c", c=CIN),
                    axis=mybir.AxisListType.X,
                    op=Add,
                )
                nc.sync.dma_start(out_dram.ap()[qs, :], ot[:])

    nc.compile()
    return nc


def _stage_inputs(times, features, W1, b1, W2, b2, W3, b3, with_corr):
    """Host-side staging: shard + precompute per-core input tensors."""
    times = np.ascontiguousarray(times, dtype=np.float32)
    features = np.ascontiguousarray(features, dtype=np.float32)
    W1 = np.asarray(W1, np.float32).reshape(1, HID)
    b1 = np.asarray(b1, np.float32).reshape(HID)
    W2 = np.asarray(W2, np.float32)
    b2 = np.asarray(b2, np.float32).reshape(HID)
    W3 = np.asarray(W3, np.float32)
    b3 = np.asarray(b3, np.float32).reshape(CIN * COUT)

    eye = np.eye(HID, dtype=np.float32)
    b1c = np.ascontiguousarray(b1[:, None])
    b2c = np.ascontiguousarray(b2[:, None])
    if with_corr:
        h2_0 = np.maximum(W2.T @ np.maximum(b1, 0.0) + b2, 0.0)
        kv0 = h2_0 @ W3
        rhs2 = np.ascontiguousarray(np.stack([b3, -kv0]).astype(np.float32))
    dd = np.arange(K)[:, None]

    in_maps = []
    for c in range(NCORES):
        b, s = divmod(c, NSH)
        gi = s * SH + np.arange(SH)
        src = gi[None, :] - 1 - dd
        m = {
            "tA": np.ascontiguousarray(np.broadcast_to(times[b, gi], (K, SH))),
            "tB": np.ascontiguousarray(times[b, np.clip(src, 0, L - 1)]),
            "mask16": (src >= 0).astype(np.float32),
            "featq": np.ascontiguousarray(
                features[b, gi].reshape(NQT, 128, CIN)
                .transpose(1, 0, 2).reshape(128, NQT * CIN)
            ),
            "W1r": W1,
            "W2": W2,
            "W3": W3,
            "b1c": b1c,
            "b2c": b2c,
            "eye": eye,
        }
        if with_corr:
            m["nvmat"] = np.ascontiguousarray(
                np.stack([np.minimum(gi, K), K - np.minimum(gi, K)])
            ).astype(np.float32)
            m["rhs2"] = rhs2
        in_maps.append(m)
    return in_maps


def kernel(times, features, W1, b1, W2, b2, W3, b3, kernel_size, **run_kwargs):
    assert int(kernel_size) == K
    assert times.shape == (BS, L) and features.shape == (BS, L, CIN)

    import os
    zero_bias = not (
        np.any(np.asarray(b1)) or np.any(np.asarray(b2)) or np.any(np.asarray(b3))
    )
    if os.environ.get("FORCE_OLD"):
        zero_bias = False
    if zero_bias:
        if "fast" not in _cache:
            _cache["fast"] = _build_fast()
        nc = _cache["fast"]
        in_maps = _stage_fast(
            times, features,
            np.asarray(W1, np.float32), np.asarray(W2, np.float32),
            np.asarray(W3, np.float32),
        )
    else:
        if ("nc", True) not in _cache:
            _cache[("nc", True)] = _build_bass(True)
        nc = _cache[("nc", True)]
        in_maps = _stage_inputs(times, features, W1, b1, W2, b2, W3, b3, True)

    res = run_bass_kernel_spmd(
        nc, in_maps, core_ids=list(range(NCORES)), **run_kwargs
    )

    out = np.empty((BS, L, COUT), np.float32)
    for c in range(NCORES):
        b, s = divmod(c, NSH)
        r = res.results[c]["out"]
        if zero_bias:
            # packed fp16 [q', t*32+o] -> fp32 [t*128+q', o]
            r = (
                r.astype(np.float32)
                .reshape(128, NQT, COUT).transpose(1, 0, 2).reshape(SH, COUT)
            )
        out[b, s * SH : (s + 1) * SH, :] = r
    if run_kwargs:
        _cache["last_results"] = res
    return out


# revision 26
# speedup vs baseline: 1.0983x; 1.0983x over previous
"""Trainium2 Bass kernel for banded continuous-conv1d (sparse_attention).

Math (per batch b, position i, K=16 band offsets d=1..K):
    dt[b,i,d] = relu(t_i - t_{i-d})           (masked where i-d < 0)
    h1 = relu(dt @ W1 + b1)                   (scalar -> 128)
    h2 = relu(h1 @ W2 + b2)                   (128 -> 128)
    kv = (h2 @ W3 + b3) masked                (128 -> 32*32)
    out[b,i,o] = sum_{d,c} feat[b,i,c] * kv[b,i,d,c,o]

Fast path (zero biases, the spec's declared fill): times are sorted so
dt >= 0 always, and relu is positively homogeneous, so the whole MLP
collapses to a rank-1 map:
    h1 = dt * relu(W1);  h2 = relu(dt * (relu(W1) @ W2)) = dt * relu(relu(W1) @ W2)
    kv = dt * w,  w = relu(relu(W1) @ W2) @ W3          (host-precomputed)
    out[b,i,o] = S[b,i] * (feat[b,i,:] @ Wt)[o]
where S[b,i] = sum_d valid * (t_i - t_{i-d}) and Wt = w.reshape(32,32).
The device computes S (banded time-sum) and feat @ Wt, then scales.
Masking is folded into the staged tA/tB tensors (zeros at invalid
entries), so no relu/mask ops are needed on device.

Nonzero biases fall back to the full per-offset MLP pipeline below
(phase-separated matmuls with rank-1 bias/mask corrections).

Sharding: 8 cores = 2 batches x 4 sequence shards of 512 positions.
"""

import sys

import numpy as np

sys.path.insert(0, "/opt/trn_rl_repo")

from concourse import bacc, bass, mybir, tile  # noqa: E402
from concourse.bass_utils import run_bass_kernel_spmd  # noqa: E402

BS, L, CIN, COUT, HID, K = 2, 2048, 32, 32, 128, 16
NCORES = 8
NSH = 4          # sequence shards per batch
SH = L // NSH    # positions per core (512)
NQT = SH // 128  # q-tiles per core (4)
F32 = mybir.dt.float32

_cache: dict = {}


def _enable_ldw_opt():
    """Let walrus dedup identical consecutive LDWEIGHTS (the default
    --enable-ldw-opt=false re-loads the stationary operand before every
    matmul)."""
    from concourse import bass_utils

    if getattr(bass_utils.run_command, "_ldw_patched", False):
        return
    orig = bass_utils.run_command

    def patched(cmd, *a, **kw):
        cmd = [
            c.replace("--enable-ldw-opt=false", "--enable-ldw-opt=true")
            if isinstance(c, str) else c
            for c in cmd
        ]
        return orig(cmd, *a, **kw)

    patched._ldw_patched = True
    bass_utils.run_command = patched


def _build_fast():
    """Collapsed zero-bias kernel: out = S * (feat @ Wt) per position.

    Note: no _enable_ldw_opt here — fp16 InstLdweights is incompatible
    with walrus --enable-ldw-opt, and the 4 matmuls all have distinct
    stationary operands so dedup buys nothing."""
    # Skip the const-pool all_engine_barrier emitted by Bass.__init__ (we
    # never read the const APs, and the runtime entry barrier already
    # synchronizes the engines) — it stalls every queue ~0.7us before the
    # first input DMA can issue. Scoped to construction only; TileContext
    # teardown barriers are untouched.
    orig_barrier = bass.Bass.all_engine_barrier

    def _skip_barrier(self, *, sem_only=False):
        pass

    bass.Bass.all_engine_barrier = _skip_barrier
    try:
        nc = bacc.Bacc("TRN2", target_bir_lowering=False, debug=False)
    finally:
        bass.Bass.all_engine_barrier = orig_barrier

    F16 = mybir.dt.float16
    # tS[q', t*17+d] = -t_{i-1-d} * valid  (d<16),  t_i * nv(i)  (d=16),
    # so S[q', t] is a single segmented reduce.
    # featW[c, 0:512] = features transposed; [c, 512:544] = Wt
    dram = {
        "tS": nc.dram_tensor("tS", [128, NQT * (K + 1)], F32,
                             kind="ExternalInput"),
        "featW": nc.dram_tensor("featW", [CIN, SH + COUT], F16,
                                kind="ExternalInput"),
    }
    # packed partition-major: out[q', t*32+o]; host unshard untangles it
    out_dram = nc.dram_tensor("out", [128, NQT * COUT], F16, kind="ExternalOutput")

    Add = mybir.AluOpType.add
    Mult = mybir.AluOpType.mult

    with tile.TileContext(nc) as tc:
        with (
            tc.tile_pool(name="const", bufs=1) as const,
            tc.tile_pool(name="work", bufs=1) as work,
            tc.tile_pool(name="ps", bufs=1, space=bass.MemorySpace.PSUM) as ps,
        ):
            qeng = {"featW": nc.sync, "tS": nc.scalar}
            sb = {}
            for name in dram:
                t = const.tile(list(dram[name].shape), dram[name].dtype, tag=name)
                qeng[name].dma_start(t[:], dram[name].ap())
                sb[name] = t

            # S[q', t] = t_i*nv - sum_d t_{i-1-d}*valid  (signs pre-staged)
            S = work.tile([128, NQT], F32, tag="S")
            nc.vector.tensor_reduce(
                S[:],
                sb["tS"][:].rearrange("p (t d) -> p t d", d=K + 1),
                axis=mybir.AxisListType.X,
                op=Add,
            )

            # fw[q, t*32+o] = (feat @ Wt)[q-tile t]; all 4 in one PSUM bank
            kvp = ps.tile([128, 512], F32, tag="kv")
            for t in range(NQT):
                qs = slice(t * 128, (t + 1) * 128)
                nc.tensor.matmul(
                    kvp[:, t * COUT : (t + 1) * COUT],
                    sb["featW"][:, qs], sb["featW"][:, SH:],
                    start=True, stop=True,
                )
            # out = S * fw in one broadcast multiply, then one packed DMA
            ot = work.tile([128, NQT * COUT], F16, tag="ot")
            nc.vector.tensor_tensor(
                ot[:].rearrange("p (t o) -> p t o", o=COUT),
                kvp[:, : NQT * COUT].rearrange("p (t o) -> p t o", o=COUT),
                S[:].unsqueeze(2).broadcast_to([128, NQT, COUT]),
                op=Mult,
            )
            nc.sync.dma_start(out_dram.ap(), ot[:])

    nc.compile()
    return nc


def _stage_fast(times, features, W1, W2, W3):
    times = np.ascontiguousarray(times, dtype=np.float32)
    features = np.ascontiguousarray(features, dtype=np.float32)
    w = np.maximum(np.maximum(W1.reshape(-1), 0.0) @ W2, 0.0) @ W3
    Wt = np.ascontiguousarray(w.reshape(CIN, COUT), np.float32)
    dd = np.arange(K)

    in_maps = []
    for c in range(NCORES):
        b, s = divmod(c, NSH)
        pos = s * SH + np.arange(SH).reshape(NQT, 128)      # [t, q']
        src = pos[:, :, None] - 1 - dd[None, None, :]       # [t, q', d]
        valid = src >= 0
        tqnv = times[b, pos] * valid.sum(-1)                # [t, q']
        tBm = -(times[b, np.clip(src, 0, L - 1)] * valid)   # [t, q', d]
        tS = (
            np.concatenate([tBm, tqnv[:, :, None]], axis=2)  # [t, q', 17]
            .transpose(1, 0, 2).reshape(128, NQT * (K + 1))
        )
        featW = np.concatenate(
            [features[b, s * SH : (s + 1) * SH].T, Wt], axis=1
        )
        in_maps.append({
            "tS": np.ascontiguousarray(tS, np.float32),
            "featW": np.ascontiguousarray(featW.astype(np.float16)),
        })
    return in_maps


# ---------------------------------------------------------------------------
# Full per-offset MLP pipeline (fallback for nonzero biases).
# ---------------------------------------------------------------------------

def _build_bass(with_corr):
    """Build + compile the SPMD single-core Bass program (identical on all
    cores; per-core behavior comes entirely from the input tensors)."""
    _enable_ldw_opt()
    nc = bacc.Bacc("TRN2", target_bir_lowering=False, debug=False)

    R32 = mybir.dt.float32r  # fp32 bits, single-pass PE mode (1 cyc/row vs 4)
    specs = [
        ("tA", (K, SH), F32),       # t_i broadcast over d rows
        ("tB", (K, SH), F32),       # t_{i-1-d}, halo-padded (clipped to t_0)
        ("mask16", (K, SH), F32),   # 1.0 where i-1-d >= 0
        ("featq", (128, NQT * CIN), F32),  # feat[q, t*32+c] (q-tile-major)
        ("W1r", (1, HID), R32),     # W1 row
        ("W2", (HID, HID), R32),
        ("W3", (HID, CIN * COUT), R32),
        ("b1c", (HID, 1), F32),
        ("b2c", (HID, 1), F32),
        ("eye", (HID, HID), R32),   # identity for the d-sum PSUM accumulation
    ]
    if with_corr:
        specs += [
            ("nvmat", (2, SH), R32),    # rows: nv, K-nv (valid-offset counts)
            ("rhs2", (2, CIN * COUT), R32),  # rows: b3, -kv0
        ]
    dram = {}
    for name, shape, dt_ in specs:
        dram[name] = nc.dram_tensor(name, list(shape), dt_, kind="ExternalInput")
    out_dram = nc.dram_tensor("out", [SH, COUT], F32, kind="ExternalOutput")

    Relu = mybir.ActivationFunctionType.Relu
    Add = mybir.AluOpType.add
    Max = mybir.AluOpType.max
    Mult = mybir.AluOpType.mult

    NW = 1024  # wide tile: 2 d-offsets side by side (2 PSUM banks)

    with tile.TileContext(nc) as tc:
        with (
            tc.tile_pool(name="const", bufs=1) as const,
            tc.tile_pool(name="work", bufs=1) as work,
            tc.tile_pool(name="h1p", bufs=8) as h1p,
            tc.tile_pool(name="h2p", bufs=8) as h2p,
            tc.tile_pool(name="stage5", bufs=2) as s5p,
            tc.tile_pool(name="ps1", bufs=2, space=bass.MemorySpace.PSUM) as ps1,
            tc.tile_pool(name="ps2", bufs=2, space=bass.MemorySpace.PSUM) as ps2,
        ):
            wzf = work.tile([HID, SH], F32, tag="wzf")
            nc.vector.memset(wzf[:], 0.0)
            wz = work.tile([HID, SH], R32, tag="wz")
            nc.vector.tensor_copy(wz[:], wzf[:])

            qeng = {
                "tA": nc.sync, "tB": nc.sync, "mask16": nc.sync,
                "W1r": nc.scalar, "b1c": nc.scalar, "W2": nc.scalar,
                "b2c": nc.scalar, "eye": nc.scalar,
                "W3": nc.gpsimd, "featq": nc.gpsimd,
                "nvmat": nc.gpsimd, "rhs2": nc.gpsimd,
            }
            sb = {}
            for name in dram:
                t = const.tile(list(dram[name].shape), dram[name].dtype, tag=name)
                qeng[name].dma_start(t[:], dram[name].ap())
                sb[name] = t

            for i in range(12):
                pw = ps1.tile([HID, NW], F32, tag="p1")
                nc.tensor.matmul(
                    pw[:, :SH], wz[:, :HID], wz[:], start=True, stop=True
                )

            dtsub = work.tile([K, SH], F32, tag="dtsub")
            nc.vector.tensor_sub(dtsub[:], sb["tA"][:], sb["tB"][:])
            dt2 = work.tile([K, SH], R32, tag="dt2")
            nc.vector.scalar_tensor_tensor(
                dt2[:], dtsub[:], 0.0, sb["mask16"][:], op0=Max, op1=Mult
            )
            dtrow = work.tile([1, K * SH], R32, tag="dtrow")
            nc.sync.dma_start(
                dtrow[:].rearrange("p (d q) -> p d q", d=K), dt2[:, :]
            )
            drows = [dtrow[:, d * SH : (d + 1) * SH] for d in range(K)]

            fexps = []
            for t in range(NQT):
                fe = s5p.tile([128, CIN * COUT], F32, tag=f"fe{t}")
                nc.gpsimd.tensor_copy(
                    fe[:].rearrange("p (o c) -> p o c", c=CIN),
                    sb["featq"][:, t * CIN : (t + 1) * CIN]
                    .unsqueeze(1)
                    .broadcast_to([128, COUT, CIN]),
                )
                fexps.append(fe)

            h1s = []
            for p in range(K // 2):
                pA = ps1.tile([HID, NW], F32, tag="p1")
                for j in range(2):
                    nc.tensor.matmul(
                        pA[:, j * SH : (j + 1) * SH], sb["W1r"][:],
                        drows[2 * p + j], start=True, stop=True,
                    )
                h1 = h1p.tile([HID, NW], R32, tag="h1")
                nc.scalar.activation(
                    h1[:, :SH], pA[:, :SH], Relu, bias=sb["b1c"][:]
                )
                nc.vector.tensor_scalar(
                    h1[:, SH:], pA[:, SH:], sb["b1c"][:], 0.0, op0=Add, op1=Max
                )
                h1s.append(h1)
            h2s = []
            for p in range(K // 2):
                pB = ps2.tile([HID, NW], F32, tag="p2")
                for j in range(2):
                    nc.tensor.matmul(
                        pB[:, j * SH : (j + 1) * SH], sb["W2"][:],
                        h1s[p][:, j * SH : (j + 1) * SH], start=True, stop=True,
                    )
                h2 = h2p.tile([HID, NW], R32, tag="h2")
                nc.vector.tensor_scalar(
                    h2[:, :SH], pB[:, :SH], sb["b2c"][:], 0.0, op0=Add, op1=Max
                )
                nc.scalar.activation(
                    h2[:, SH:], pB[:, SH:], Relu, bias=sb["b2c"][:]
                )
                h2s.append(h2)
            pHw = ps1.tile([HID, NW], F32, tag="p1")
            pH = pHw[:, :SH]
            n = 0
            for p in range(K // 2):
                for j in range(2):
                    nc.tensor.matmul(
                        pH, sb["eye"][:], h2s[p][:, j * SH : (j + 1) * SH],
                        start=(n == 0), stop=(n == K - 1),
                    )
                    n += 1

            Hs = work.tile([HID, SH], R32, tag="Hs")
            nc.vector.tensor_copy(Hs[:], pH)

            CO = CIN * COUT
            for t in range(NQT):
                qs = slice(t * 128, (t + 1) * 128)
                kv = (ps2 if t % 2 == 0 else ps1).tile(
                    [128, CO], F32, tag="p2" if t % 2 == 0 else "p1"
                )
                for half in range(2):
                    hs = slice(half * 512, half * 512 + 512)
                    nc.tensor.matmul(
                        kv[:, hs], Hs[:, qs], sb["W3"][:, hs],
                        start=True, stop=not with_corr,
                    )
                if with_corr:
                    for half in range(2):
                        hs = slice(half * 512, half * 512 + 512)
                        nc.tensor.matmul(
                            kv[:, hs], sb["nvmat"][:, qs], sb["rhs2"][:, hs],
                            start=False, stop=True,
                        )
                prod = s5p.tile([128, CO], F32, tag="prod")
                kvT = kv[:].rearrange("p (c o) -> p o c", o=COUT)
                prodv = prod[:].rearrange("p (o c) -> p o c", c=CIN)
                fev = fexps[t][:].rearrange("p (o c) -> p o c", c=CIN)
                if t < 1:
                    kvs = s5p.tile([128, CO], F32, tag="kvs")
                    nc.scalar.copy(kvs[:], kv[:])
                    nc.gpsimd.tensor_tensor(
                        prodv,
                        kvs[:].rearrange("p (c o) -> p o c", o=COUT),
                        fev, op=Mult,
                    )
                else:
                    nc.vector.tensor_tensor(prodv, kvT, fev, op=Mult)
                ot = s5p.tile([128, COUT], F32, tag="ot")
                nc.vector.tensor_reduce(
                    ot[:],
                    prod[:].rearrange("p (o c) -> p o c", c=CIN),
                    axis=mybir.AxisListType.X,
                    op=Add,
                )
                nc.sync.dma_start(out_dram.ap()[qs, :], ot[:])

    nc.compile()
    return nc


def _stage_inputs(times, features, W1, b1, W2, b2, W3, b3, with_corr):
    """Host-side staging: shard + precompute per-core input tensors."""
    times = np.ascontiguousarray(times, dtype=np.float32)
    features = np.ascontiguousarray(features, dtype=np.float32)
    W1 = np.asarray(W1, np.float32).reshape(1, HID)
    b1 = np.asarray(b1, np.float32).reshape(HID)
    W2 = np.asarray(W2, np.float32)
    b2 = np.asarray(b2, np.float32).reshape(HID)
    W3 = np.asarray(W3, np.float32)
    b3 = np.asarray(b3, np.float32).reshape(CIN * COUT)

    eye = np.eye(HID, dtype=np.float32)
    b1c = np.ascontiguousarray(b1[:, None])
    b2c = np.ascontiguousarray(b2[:, None])
    if with_corr:
        h2_0 = np.maximum(W2.T @ np.maximum(b1, 0.0) + b2, 0.0)
        kv0 = h2_0 @ W3
        rhs2 = np.ascontiguousarray(np.stack([b3, -kv0]).astype(np.float32))
    dd = np.arange(K)[:, None]

    in_maps = []
    for c in range(NCORES):
        b, s = divmod(c, NSH)
        gi = s * SH + np.arange(SH)
        src = gi[None, :] - 1 - dd
        m = {
            "tA": np.ascontiguousarray(np.broadcast_to(times[b, gi], (K, SH))),
            "tB": np.ascontiguousarray(times[b, np.clip(src, 0, L - 1)]),
            "mask16": (src >= 0).astype(np.float32),
            "featq": np.ascontiguousarray(
                features[b, gi].reshape(NQT, 128, CIN)
                .transpose(1, 0, 2).reshape(128, NQT * CIN)
            ),
            "W1r": W1,
            "W2": W2,
            "W3": W3,
            "b1c": b1c,
            "b2c": b2c,
            "eye": eye,
        }
        if with_corr:
            m["nvmat"] = np.ascontiguousarray(
                np.stack([np.minimum(gi, K), K - np.minimum(gi, K)])
            ).astype(np.float32)
            m["rhs2"] = rhs2
        in_maps.append(m)
    return in_maps


def kernel(times, features, W1, b1, W2, b2, W3, b3, kernel_size, **run_kwargs):
    assert int(kernel_size) == K
    assert times.shape == (BS, L) and features.shape == (BS, L, CIN)

    import os
    zero_bias = not (
        np.any(np.asarray(b1)) or np.any(np.asarray(b2)) or np.any(np.asarray(b3))
    )
    if os.environ.get("FORCE_OLD"):
        zero_bias = False
    if zero_bias:
        if "fast" not in _cache:
            _cache["fast"] = _build_fast()
        nc = _cache["fast"]
        in_maps = _stage_fast(
            times, features,
            np.asarray(W1, np.float32), np.asarray(W2, np.float32),
            np.asarray(W3, np.float32),
        )
    else:
        if ("nc", True) not in _cache:
            _cache[("nc", True)] = _build_bass(True)
        nc = _cache[("nc", True)]
        in_maps = _stage_inputs(times, features, W1, b1, W2, b2, W3, b3, True)

    res = run_bass_kernel_spmd(
        nc, in_maps, core_ids=list(range(NCORES)), **run_kwargs
    )

    out = np.empty((BS, L, COUT), np.float32)
    for c in range(NCORES):
        b, s = divmod(c, NSH)
        r = res.results[c]["out"]
        if zero_bias:
            # packed fp16 [q', t*32+o] -> fp32 [t*128+q', o]
            r = (
                r.astype(np.float32)
                .reshape(128, NQT, COUT).transpose(1, 0, 2).reshape(SH, COUT)
            )
        out[b, s * SH : (s + 1) * SH, :] = r
    if run_kwargs:
        _cache["last_results"] = res
    return out
